# Initial kernel scaffold
#
"""Binarized 4-layer MLP on 8 Trainium2 NeuronCores.

Math (from the reference):
    h = x.transpose(0,2,1).reshape(8192, 512)          rows = (n, t), cols = f
    for l in 1..3:  h = sign(batchnorm(h @ sign(Wl).T, gl, bl))
    y = ((h @ sign(W4).T) * scale).reshape(16,512,512).transpose(0,2,1)
(The depthwise conv in the reference is dead code - its result is discarded.)

Strategy:
  - Data parallel over the 8192 rows: core c owns rows of batch elements
    n = 2c, 2c+1 (1024 rows).
  - Activations live transposed on chip: [hidden partition, ktile, row] so
    every matmul is lhsT.T @ rhs with both operands in natural layout and
    BN stats are free-dim reductions (bn_stats) + per-partition broadcasts.
  - After each BN+sign the activations are exactly {-1,0,+1}: layers 2-4 run
    in fp8(e4m3) with DoubleRow at exact integer accumulation in fp32 PSUM.
  - Layer 1 runs in bf16 with x split hi/lo (x = hi + lo) which preserves
    ~2^-17 relative accuracy - inside the fp32 round-off envelope of the
    reference.
  - BatchNorm needs full-batch stats: each core computes per-hidden
    (mean, E[x^2]) over its 1024 rows (bn_stats/bn_aggr), AllGathers the
    8 partials (16KB), and reduces locally.  sign(g*(h-mu)*rsqrt(var+eps)+b)
    == sign(h - thr) with thr = mu - (b/g)*sqrt(var+eps) for g > 0.
  - Weights are binarized + laid out host-side (layout prep only touches
    sign/transpose/dtype).
"""

import numpy as np
import ml_dtypes

import concourse.bacc as bacc
import concourse.tile as tile
import concourse.mybir as mybir
from concourse.bass_utils import run_bass_kernel_spmd

# ---- problem constants (hardcoded per the grading contract) ----
NB, F, T, H = 16, 512, 512, 2048
CORES = 8
NPC = NB // CORES          # batch elements per core = 2
RPC = NPC * T              # rows per core = 1024
C = RPC // 512             # 512-row chunks per core = 2
KT1 = F // 128             # k-tiles layer 1 = 4
KT = H // 128              # k-tiles layers 2-4 = 16
MT_H = H // 128            # out tiles layers 1-3 = 16
MT_F = F // 128            # out tiles layer 4 = 4
EPS = 1e-5

F32 = mybir.dt.float32
BF16 = mybir.dt.bfloat16
FP8 = mybir.dt.float8e4
BF16_NP = ml_dtypes.bfloat16
FP8_NP = ml_dtypes.float8_e4m3
SQRT = mybir.ActivationFunctionType.Sqrt
DR = mybir.MatmulPerfMode.DoubleRow

_CACHE = {}


def _build_nc():
    nc = bacc.Bacc(
        "TRN2",
        target_bir_lowering=False,
        debug=False,
        enable_asserts=False,
        num_devices=CORES,
    )
    xhi_d = nc.dram_tensor("xhi", [128, KT1, C, 512], BF16, kind="ExternalInput")
    xlo_d = nc.dram_tensor("xlo", [128, KT1, C, 512], BF16, kind="ExternalInput")
    w1_d = nc.dram_tensor("w1q", [MT_H, 128, KT1, 128], BF16, kind="ExternalInput")
    w2_d = nc.dram_tensor("w2q", [MT_H, 128, KT, 128], FP8, kind="ExternalInput")
    w3_d = nc.dram_tensor("w3q", [MT_H, 128, KT, 128], FP8, kind="ExternalInput")
    w4_d = nc.dram_tensor("w4q", [MT_F, 128, KT, 128], FP8, kind="ExternalInput")
    bg_d = nc.dram_tensor("bgq", [128, 3, KT], F32, kind="ExternalInput")
    sc_d = nc.dram_tensor("scq", [128, MT_F], F32, kind="ExternalInput")
    y_d = nc.dram_tensor("y", [128, MT_F, C, 512], F32, kind="ExternalOutput")

    with tile.TileContext(nc) as tc:
        with (
            tc.tile_pool(name="persist", bufs=1) as sb,
            tc.tile_pool(name="wts", bufs=6) as wpool,
            tc.tile_pool(name="psum", bufs=8, space="PSUM") as pp,
            tc.tile_pool(name="dram", bufs=1, space="DRAM") as dp,
        ):
            # persistent SBUF tensors
            xh = sb.tile([128, KT1, C, 512], BF16, name="xh")
            xl = sb.tile([128, KT1, C, 512], BF16, name="xl")
            hpre = sb.tile([128, KT, C, 512], F32, name="hpre")
            act_a = sb.tile([128, KT, C, 512], FP8, name="act_a")
            act_b = sb.tile([128, KT, C, 512], FP8, name="act_b")
            stats6 = sb.tile([128, KT, C, 6], F32, name="stats6")
            locms = sb.tile([128, KT, 2], F32, name="locms")
            part = sb.tile([128, 2, KT], F32, name="part")
            gath = sb.tile([128, CORES, 2 * KT], F32, name="gath")
            bgs = sb.tile([128, 3, KT], F32, name="bgs")
            scs = sb.tile([128, MT_F], F32, name="scs")
            yout = sb.tile([128, MT_F, C, 512], F32, name="yout")

            nc.sync.dma_start(xh[:], xhi_d.ap())
            nc.sync.dma_start(xl[:], xlo_d.ap())
            nc.sync.dma_start(bgs[:], bg_d.ap())
            nc.sync.dma_start(scs[:], sc_d.ap())

            def mm_layer(w_dram, wdt, wkt, mt, rhs_list, dr):
                """Matmuls of one layer; fills hpre[:, :mt] and stats6[:, :mt]."""
                for m in range(mt):
                    wt = wpool.tile([128, wkt, 128], wdt, name="wt", tag="wt")
                    nc.sync.dma_start(wt[:], w_dram.ap()[m])
                    for c in range(C):
                        ps = pp.tile([128, 512], F32, name="ps", tag="ps")
                        if dr:
                            nk = wkt // 2
                            for kk in range(nk):
                                nc.tensor.matmul(
                                    ps[:],
                                    lhsT=wt[:, 2 * kk : 2 * kk + 2, :],
                                    rhs=rhs_list[0][:, 2 * kk : 2 * kk + 2, c, :],
                                    start=(kk == 0),
                                    stop=(kk == nk - 1),
                                    perf_mode=DR,
                                )
                        else:
                            ns = wkt * len(rhs_list)
                            s = 0
                            for kk in range(wkt):
                                for rhs in rhs_list:
                                    nc.tensor.matmul(
                                        ps[:],
                                        lhsT=wt[:, kk, :],
                                        rhs=rhs[:, kk, c, :],
                                        start=(s == 0),
                                        stop=(s == ns - 1),
                                    )
                                    s += 1
                        nc.vector.bn_stats(stats6[:, m, c, :], ps[:])
                        nc.scalar.copy(hpre[:, m, c, :], ps[:])

            def bn_sign(li, acts_out):
                """Cross-core BN stats + sign; reads hpre/stats6, writes acts_out."""
                for m in range(MT_H):
                    nc.vector.bn_aggr(
                        locms[:, m, :],
                        stats6[:, m, :, :].rearrange("p a b -> p (a b)"),
                    )
                tmp16 = sb.tile([128, KT], F32, name=f"tmp16_{li}", tag="tmp16")
                nc.vector.tensor_copy(part[:, 0, :], locms[:, :, 0])
                nc.vector.tensor_mul(tmp16[:], locms[:, :, 0], locms[:, :, 0])
                nc.vector.tensor_add(part[:, 1, :], locms[:, :, 1], tmp16[:])

                ccin = dp.tile([128, 2 * KT], F32, name=f"ccin{li}")
                ccout = dp.tile(
                    [CORES * 128, 2 * KT], F32, name=f"ccout{li}", addr_space="Shared"
                )
                nc.sync.dma_start(ccin[:], part[:])
                nc.gpsimd.collective_compute(
                    "AllGather",
                    mybir.AluOpType.bypass,
                    replica_groups=[list(range(CORES))],
                    ins=[ccin.opt()],
                    outs=[ccout.opt()],
                )
                nc.sync.dma_start(
                    gath[:], ccout[:].rearrange("(r p) n -> p r n", p=128)
                )
                t4 = sb.tile([128, 4, 2 * KT], F32, name=f"t4_{li}", tag="t4")
                t2 = sb.tile([128, 2, 2 * KT], F32, name=f"t2_{li}", tag="t2")
                t1 = sb.tile([128, 2 * KT], F32, name=f"t1_{li}", tag="t1")
                nc.vector.tensor_add(t4[:], gath[:, 0:4, :], gath[:, 4:8, :])
                nc.vector.tensor_add(t2[:], t4[:, 0:2, :], t4[:, 2:4, :])
                nc.vector.tensor_add(t1[:], t2[:, 0, :], t2[:, 1, :])
                ex = sb.tile([128, KT], F32, name=f"ex_{li}", tag="ex")
                exx = sb.tile([128, KT], F32, name=f"exx_{li}", tag="exx")
                var = sb.tile([128, KT], F32, name=f"var_{li}", tag="var")
                std = sb.tile([128, KT], F32, name=f"std_{li}", tag="std")
                nthr = sb.tile([128, KT], F32, name=f"nthr_{li}", tag="nthr")
                inv = 1.0 / CORES
                nc.vector.tensor_scalar_mul(ex[:], t1[:, 0:KT], inv)
                nc.vector.tensor_scalar_mul(exx[:], t1[:, KT : 2 * KT], inv)
                nc.vector.tensor_mul(var[:], ex[:], ex[:])
                nc.vector.tensor_sub(var[:], exx[:], var[:])
                nc.scalar.activation(std[:], var[:], SQRT, bias=EPS)
                nc.vector.tensor_mul(nthr[:], bgs[:, li, :], std[:])
                nc.vector.tensor_sub(nthr[:], nthr[:], ex[:])
                for m in range(MT_H):
                    nc.scalar.sign(
                        acts_out[:, m, :, :], hpre[:, m, :, :], bias=nthr[:, m : m + 1]
                    )

            # layer 1 (bf16 hi+lo) -> BN1 -> act_a
            mm_layer(w1_d, BF16, KT1, MT_H, [xh, xl], dr=False)
            bn_sign(0, act_a)
            # layer 2 (fp8) -> BN2 -> act_b
            mm_layer(w2_d, FP8, KT, MT_H, [act_a], dr=True)
            bn_sign(1, act_b)
            # layer 3 (fp8) -> BN3 -> act_a
            mm_layer(w3_d, FP8, KT, MT_H, [act_b], dr=True)
            bn_sign(2, act_a)
            # layer 4 (fp8) + per-feature scale
            for m in range(MT_F):
                wt = wpool.tile([128, KT, 128], FP8, name="wt4", tag="wt")
                nc.sync.dma_start(wt[:], w4_d.ap()[m])
                for c in range(C):
                    ps = pp.tile([128, 512], F32, name="ps4", tag="ps")
                    for kk in range(KT // 2):
                        nc.tensor.matmul(
                            ps[:],
                            lhsT=wt[:, 2 * kk : 2 * kk + 2, :],
                            rhs=act_a[:, 2 * kk : 2 * kk + 2, c, :],
                            start=(kk == 0),
                            stop=(kk == KT // 2 - 1),
                            perf_mode=DR,
                        )
                    nc.scalar.mul(yout[:, m, c, :], ps[:], mul=scs[:, m : m + 1])
            nc.sync.dma_start(y_d.ap(), yout[:])

    nc.compile()
    return nc


def _get_nc():
    if "nc" not in _CACHE:
        _CACHE["nc"] = _build_nc()
    return _CACHE["nc"]


def _wq(W, np_dt):
    """sign(W).T laid out [mt, 128, kt, 128] = (out tile, in%128, in//128, out%128)."""
    Wt = np.sign(np.asarray(W, np.float32)).T
    IN, OUT = Wt.shape
    kt, mt = IN // 128, OUT // 128
    return np.ascontiguousarray(
        Wt.reshape(kt, 128, mt, 128).transpose(2, 1, 0, 3).astype(np_dt)
    )


def _prep_in_maps(inputs):
    x = np.asarray(inputs["x"], np.float32)
    xhi = x.astype(BF16_NP)
    xlo = (x - xhi.astype(np.float32)).astype(BF16_NP)

    w1q = _wq(inputs["W1"], BF16_NP)
    w2q = _wq(inputs["W2"], FP8_NP)
    w3q = _wq(inputs["W3"], FP8_NP)
    w4q = _wq(inputs["W4"], FP8_NP)

    def _pk(v):  # (2048,) -> [128, 16]
        return np.ascontiguousarray(np.asarray(v, np.float32).reshape(KT, 128).T)

    bgq = np.stack(
        [
            _pk(np.where(inputs[g] != 0, inputs[b] / inputs[g], 0.0))
            for g, b in (("g1", "b1"), ("g2", "b2"), ("g3", "b3"))
        ],
        axis=1,
    ).astype(np.float32)
    scq = np.ascontiguousarray(
        np.asarray(inputs["scale"], np.float32).reshape(MT_F, 128).T
    )

    def _xq(a, c):  # per-core x slice -> [128, KT1, NPC, 512]
        s = a[NPC * c : NPC * (c + 1)]  # (2, 512, 512) = (n, f, t)
        return np.ascontiguousarray(
            s.reshape(NPC, KT1, 128, T).transpose(2, 1, 0, 3)
        )

    in_maps = []
    for c in range(CORES):
        in_maps.append(
            {
                "xhi": _xq(xhi, c),
                "xlo": _xq(xlo, c),
                "w1q": w1q,
                "w2q": w2q,
                "w3q": w3q,
                "w4q": w4q,
                "bgq": bgq,
                "scq": scq,
            }
        )
    return in_maps


def _assemble(results):
    y = np.empty((NB, F, T), np.float32)
    for c in range(CORES):
        r = results[c]["y"]  # [128, MT_F, C, 512]
        y[NPC * c : NPC * (c + 1)] = (
            r.transpose(2, 1, 0, 3).reshape(NPC, F, T)
        )
    return y


def kernel(**inputs):
    nc = _get_nc()
    in_maps = _prep_in_maps(inputs)
    res = run_bass_kernel_spmd(nc, in_maps, core_ids=list(range(CORES)))
    return _assemble(res.results)


if __name__ == "__main__":
    rng = np.random.default_rng(0)
    ins = dict(
        x=rng.standard_normal((NB, F, T), np.float32),
        conv_w=rng.standard_normal((F, 1, 5), np.float32),
        W1=rng.standard_normal((H, F), np.float32),
        g1=np.ones(H, np.float32), b1=np.zeros(H, np.float32),
        W2=rng.standard_normal((H, H), np.float32),
        g2=np.ones(H, np.float32), b2=np.zeros(H, np.float32),
        W3=rng.standard_normal((H, H), np.float32),
        g3=np.ones(H, np.float32), b3=np.zeros(H, np.float32),
        W4=rng.standard_normal((F, H), np.float32),
        scale=np.ones(F, np.float32),
    )
    out = kernel(**ins)
    print(out.shape, out.dtype)


# revision 8
# speedup vs baseline: 10.1970x; 10.1970x over previous
"""Binarized 4-layer MLP on 8 Trainium2 NeuronCores.

Math (from the reference):
    h = x.transpose(0,2,1).reshape(8192, 512)          rows = (n, t), cols = f
    for l in 1..3:  h = sign(batchnorm(h @ sign(Wl).T, gl, bl))
    y = ((h @ sign(W4).T) * scale).reshape(16,512,512).transpose(0,2,1)
(The depthwise conv in the reference is dead code - its result is discarded.)

Strategy:
  - Data parallel over the 8192 rows: core c owns rows of batch elements
    n = 2c, 2c+1 (1024 rows).
  - Activations live transposed on chip: [hidden partition, ktile, row] so
    every matmul is lhsT.T @ rhs with both operands in natural layout and
    BN stats are free-dim reductions (bn_stats) + per-partition broadcasts.
  - After each BN+sign the activations are exactly {-1,0,+1}: layers 2-4 run
    in fp8(e4m3) with DoubleRow at exact integer accumulation in fp32 PSUM.
  - Layer 1 runs in bf16 with x split hi/lo (x = hi + lo) which preserves
    ~2^-17 relative accuracy - inside the fp32 round-off envelope of the
    reference.
  - BatchNorm needs full-batch stats: each core computes per-hidden
    (mean, E[x^2]) over its 1024 rows (bn_stats/bn_aggr), AllGathers the
    8 partials (16KB), and reduces locally.  sign(g*(h-mu)*rsqrt(var+eps)+b)
    == sign(h - thr) with thr = mu - (b/g)*sqrt(var+eps) for g > 0.
  - Weights are binarized + laid out host-side (layout prep only touches
    sign/transpose/dtype).
"""

import numpy as np
import ml_dtypes

import concourse.bacc as bacc
import concourse.tile as tile
import concourse.mybir as mybir
from concourse.bass_utils import run_bass_kernel_spmd

# ---- problem constants (hardcoded per the grading contract) ----
NB, F, T, H = 16, 512, 512, 2048
CORES = 8
NPC = NB // CORES          # batch elements per core = 2
RPC = NPC * T              # rows per core = 1024
C = RPC // 512             # 512-row chunks per core = 2
KT1 = F // 128             # k-tiles layer 1 = 4
KT = H // 128              # k-tiles layers 2-4 = 16
MT_H = H // 128            # out tiles layers 1-3 = 16
MT_F = F // 128            # out tiles layer 4 = 4
EPS = 1e-5

F32 = mybir.dt.float32
BF16 = mybir.dt.bfloat16
FP16 = mybir.dt.float16
FP8 = mybir.dt.float8e4
BF16_NP = ml_dtypes.bfloat16
FP8_NP = ml_dtypes.float8_e4m3
SQRT = mybir.ActivationFunctionType.Sqrt
DR = mybir.MatmulPerfMode.DoubleRow

_CACHE = {}


def _build_nc(n_cores=CORES, with_cc=True):
    nc = bacc.Bacc(
        "TRN2",
        target_bir_lowering=False,
        debug=False,
        enable_asserts=False,
        num_devices=n_cores,
    )
    xhi_d = nc.dram_tensor("xhi", [128, KT1, C, 512], FP16, kind="ExternalInput")
    xlo_d = nc.dram_tensor("xlo", [128, KT1, C, 512], FP16, kind="ExternalInput")
    w1_d = nc.dram_tensor("w1q", [MT_H, 128, KT1, 128], FP16, kind="ExternalInput")
    w2_d = nc.dram_tensor("w2q", [MT_H, 128, KT, 128], FP8, kind="ExternalInput")
    w3_d = nc.dram_tensor("w3q", [MT_H, 128, KT, 128], FP8, kind="ExternalInput")
    w4_d = nc.dram_tensor("w4q", [MT_F, 128, KT, 128], FP8, kind="ExternalInput")
    bg_d = nc.dram_tensor("bgq", [128, 3, KT], F32, kind="ExternalInput")
    sc_d = nc.dram_tensor("scq", [128, MT_F], F32, kind="ExternalInput")
    y_d = nc.dram_tensor("y", [128, MT_F, C, 512], F32, kind="ExternalOutput")

    with tile.TileContext(nc) as tc:
        with (
            tc.tile_pool(name="persist", bufs=1) as sb,
            tc.tile_pool(name="wts", bufs=6) as wpool,
            tc.tile_pool(name="psum", bufs=8, space="PSUM") as pp,
            tc.tile_pool(name="dram", bufs=1, space="DRAM") as dp,
        ):
            # persistent SBUF tensors
            xh = sb.tile([128, KT1, C, 512], FP16, name="xh")
            xl = sb.tile([128, KT1, C, 512], FP16, name="xl")
            hpre = sb.tile([128, KT, C, 512], F32, name="hpre")
            act_a = sb.tile([128, KT, C, 512], FP8, name="act_a")
            act_b = sb.tile([128, KT, C, 512], FP8, name="act_b")
            stats6 = sb.tile([128, KT, C, 6], F32, name="stats6")
            locms = sb.tile([128, KT, 2], F32, name="locms")
            part = sb.tile([128, 2, KT], F32, name="part")
            gath = sb.tile([128, CORES, 2 * KT], F32, name="gath")
            bgs = sb.tile([128, 3, KT], F32, name="bgs")
            scs = sb.tile([128, MT_F], F32, name="scs")
            yout = sb.tile([128, MT_F, C, 512], F32, name="yout")
            epst = sb.tile([128, 1], F32, name="epst")
            nc.vector.memset(epst[:], EPS)

            nc.sync.dma_start(xh[:], xhi_d.ap())
            nc.sync.dma_start(xl[:], xlo_d.ap())
            nc.sync.dma_start(bgs[:], bg_d.ap())
            nc.sync.dma_start(scs[:], sc_d.ap())

            def mm_layer(w_dram, wdt, wkt, mt, rhs_list, dr):
                """Matmuls of one layer; fills hpre[:, :mt] and stats6[:, :mt]."""
                for m in range(mt):
                    wt = wpool.tile([128, wkt, 128], wdt, name="wt", tag="wt")
                    nc.sync.dma_start(wt[:], w_dram.ap()[m])
                    for c in range(C):
                        ps = pp.tile([128, 512], F32, name="ps", tag="ps")
                        if dr:
                            nk = wkt // 2
                            for kk in range(nk):
                                nc.tensor.matmul(
                                    ps[:],
                                    lhsT=wt[:, 2 * kk : 2 * kk + 2, :],
                                    rhs=rhs_list[0][:, 2 * kk : 2 * kk + 2, c, :],
                                    start=(kk == 0),
                                    stop=(kk == nk - 1),
                                    perf_mode=DR,
                                )
                        else:
                            ns = wkt * len(rhs_list)
                            s = 0
                            for kk in range(wkt):
                                for rhs in rhs_list:
                                    nc.tensor.matmul(
                                        ps[:],
                                        lhsT=wt[:, kk, :],
                                        rhs=rhs[:, kk, c, :],
                                        start=(s == 0),
                                        stop=(s == ns - 1),
                                    )
                                    s += 1
                        nc.vector.bn_stats(stats6[:, m, c, :], ps[:])
                        nc.scalar.copy(hpre[:, m, c, :], ps[:])

            def bn_sign(li, acts_out):
                """Cross-core BN stats + sign; reads hpre/stats6, writes acts_out."""
                for m in range(MT_H):
                    nc.vector.bn_aggr(
                        locms[:, m, :],
                        stats6[:, m, :, :].rearrange("p a b -> p (a b)"),
                    )
                tmp16 = sb.tile([128, KT], F32, name=f"tmp16_{li}", tag="tmp16")
                nc.vector.tensor_copy(part[:, 0, :], locms[:, :, 0])
                nc.vector.tensor_mul(tmp16[:], locms[:, :, 0], locms[:, :, 0])
                nc.vector.tensor_add(part[:, 1, :], locms[:, :, 1], tmp16[:])

                ex = sb.tile([128, KT], F32, name=f"ex_{li}", tag="ex")
                exx = sb.tile([128, KT], F32, name=f"exx_{li}", tag="exx")
                var = sb.tile([128, KT], F32, name=f"var_{li}", tag="var")
                std = sb.tile([128, KT], F32, name=f"std_{li}", tag="std")
                nthr = sb.tile([128, KT], F32, name=f"nthr_{li}", tag="nthr")
                if with_cc:
                    ccin = dp.tile([128, 2 * KT], F32, name=f"ccin{li}")
                    ccout = dp.tile(
                        [n_cores * 128, 2 * KT], F32,
                        name=f"ccout{li}", addr_space="Shared",
                    )
                    nc.sync.dma_start(ccin[:], part[:])
                    nc.gpsimd.collective_compute(
                        "AllGather",
                        mybir.AluOpType.bypass,
                        replica_groups=[list(range(n_cores))],
                        ins=[ccin.opt()],
                        outs=[ccout.opt()],
                    )
                    nc.sync.dma_start(
                        gath[:], ccout[:].rearrange("(r p) n -> p r n", p=128)
                    )
                    t4 = sb.tile([128, 4, 2 * KT], F32, name=f"t4_{li}", tag="t4")
                    t2 = sb.tile([128, 2, 2 * KT], F32, name=f"t2_{li}", tag="t2")
                    t1 = sb.tile([128, 2 * KT], F32, name=f"t1_{li}", tag="t1")
                    nc.vector.tensor_add(t4[:], gath[:, 0:4, :], gath[:, 4:8, :])
                    nc.vector.tensor_add(t2[:], t4[:, 0:2, :], t4[:, 2:4, :])
                    nc.vector.tensor_add(t1[:], t2[:, 0, :], t2[:, 1, :])
                    srcmean, srcexx = t1[:, 0:KT], t1[:, KT : 2 * KT]
                    inv = 1.0 / n_cores
                else:
                    srcmean, srcexx = part[:, 0, :], part[:, 1, :]
                    inv = 1.0
                nc.vector.tensor_scalar_mul(ex[:], srcmean, inv)
                nc.vector.tensor_scalar_mul(exx[:], srcexx, inv)
                nc.vector.tensor_mul(var[:], ex[:], ex[:])
                nc.vector.tensor_sub(var[:], exx[:], var[:])
                nc.vector.tensor_scalar_max(var[:], var[:], 0.0)
                nc.scalar.activation(std[:], var[:], SQRT, bias=epst[:])
                nc.vector.tensor_mul(nthr[:], bgs[:, li, :], std[:])
                nc.vector.tensor_sub(nthr[:], nthr[:], ex[:])
                for m in range(MT_H):
                    nc.scalar.sign(
                        acts_out[:, m, :, :], hpre[:, m, :, :], bias=nthr[:, m : m + 1]
                    )

            # layer 1 (bf16 hi+lo) -> BN1 -> act_a
            mm_layer(w1_d, FP16, KT1, MT_H, [xh, xl], dr=False)
            bn_sign(0, act_a)
            # layer 2 (fp8) -> BN2 -> act_b
            mm_layer(w2_d, FP8, KT, MT_H, [act_a], dr=True)
            bn_sign(1, act_b)
            # layer 3 (fp8) -> BN3 -> act_a
            mm_layer(w3_d, FP8, KT, MT_H, [act_b], dr=True)
            bn_sign(2, act_a)
            # layer 4 (fp8) + per-feature scale
            for m in range(MT_F):
                wt = wpool.tile([128, KT, 128], FP8, name="wt4", tag="wt")
                nc.sync.dma_start(wt[:], w4_d.ap()[m])
                for c in range(C):
                    ps = pp.tile([128, 512], F32, name="ps4", tag="ps")
                    for kk in range(KT // 2):
                        nc.tensor.matmul(
                            ps[:],
                            lhsT=wt[:, 2 * kk : 2 * kk + 2, :],
                            rhs=act_a[:, 2 * kk : 2 * kk + 2, c, :],
                            start=(kk == 0),
                            stop=(kk == KT // 2 - 1),
                            perf_mode=DR,
                        )
                    nc.scalar.mul(yout[:, m, c, :], ps[:], mul=scs[:, m : m + 1])
            nc.sync.dma_start(y_d.ap(), yout[:])

    nc.compile()
    return nc


def _get_nc():
    if "nc" not in _CACHE:
        _CACHE["nc"] = _build_nc()
    return _CACHE["nc"]


def _wq(W, np_dt):
    """sign(W).T laid out [mt, 128, kt, 128] = (out tile, in%128, in//128, out%128)."""
    Wt = np.sign(np.asarray(W, np.float32)).T
    IN, OUT = Wt.shape
    kt, mt = IN // 128, OUT // 128
    return np.ascontiguousarray(
        Wt.reshape(kt, 128, mt, 128).transpose(2, 1, 0, 3).astype(np_dt)
    )


def _prep_in_maps(inputs):
    x = np.asarray(inputs["x"], np.float32)
    xhi = x.astype(np.float16)
    xlo = (x - xhi.astype(np.float32)).astype(np.float16)

    w1q = _wq(inputs["W1"], np.float16)
    w2q = _wq(inputs["W2"], FP8_NP)
    w3q = _wq(inputs["W3"], FP8_NP)
    w4q = _wq(inputs["W4"], FP8_NP)

    def _pk(v):  # (2048,) -> [128, 16]
        return np.ascontiguousarray(np.asarray(v, np.float32).reshape(KT, 128).T)

    bgq = np.stack(
        [
            _pk(np.where(inputs[g] != 0, inputs[b] / inputs[g], 0.0))
            for g, b in (("g1", "b1"), ("g2", "b2"), ("g3", "b3"))
        ],
        axis=1,
    ).astype(np.float32)
    scq = np.ascontiguousarray(
        np.asarray(inputs["scale"], np.float32).reshape(MT_F, 128).T
    )

    def _xq(a, c):  # per-core x slice -> [128, KT1, NPC, 512]
        s = a[NPC * c : NPC * (c + 1)]  # (2, 512, 512) = (n, f, t)
        return np.ascontiguousarray(
            s.reshape(NPC, KT1, 128, T).transpose(2, 1, 0, 3)
        )

    in_maps = []
    for c in range(CORES):
        in_maps.append(
            {
                "xhi": _xq(xhi, c),
                "xlo": _xq(xlo, c),
                "w1q": w1q,
                "w2q": w2q,
                "w3q": w3q,
                "w4q": w4q,
                "bgq": bgq,
                "scq": scq,
            }
        )
    return in_maps


def _assemble(results):
    y = np.empty((NB, F, T), np.float32)
    for c in range(CORES):
        r = results[c]["y"]  # [128, MT_F, C, 512]
        y[NPC * c : NPC * (c + 1)] = (
            r.transpose(2, 1, 0, 3).reshape(NPC, F, T)
        )
    return y


def _valid(y, inputs):
    """Catches the (rare) garbage first execution after NEFF load: outputs
    are sums of <=2048 terms of +-1 times scale, so any non-finite value or
    magnitude above that bound means the run must be retried."""
    bound = 2048.0 * max(1.0, float(np.abs(inputs["scale"]).max())) * 1.001
    return np.isfinite(y).all() and float(np.abs(y).max()) <= bound


def kernel(**inputs):
    nc = _get_nc()
    in_maps = _prep_in_maps(inputs)
    for _ in range(3):
        res = run_bass_kernel_spmd(nc, in_maps, core_ids=list(range(CORES)))
        y = _assemble(res.results)
        if _valid(y, inputs):
            return y
    return y


if __name__ == "__main__":
    rng = np.random.default_rng(0)
    ins = dict(
        x=rng.standard_normal((NB, F, T), np.float32),
        conv_w=rng.standard_normal((F, 1, 5), np.float32),
        W1=rng.standard_normal((H, F), np.float32),
        g1=np.ones(H, np.float32), b1=np.zeros(H, np.float32),
        W2=rng.standard_normal((H, H), np.float32),
        g2=np.ones(H, np.float32), b2=np.zeros(H, np.float32),
        W3=rng.standard_normal((H, H), np.float32),
        g3=np.ones(H, np.float32), b3=np.zeros(H, np.float32),
        W4=rng.standard_normal((F, H), np.float32),
        scale=np.ones(F, np.float32),
    )
    out = kernel(**ins)
    print(out.shape, out.dtype)


# revision 21
# speedup vs baseline: 17.2410x; 1.6908x over previous
"""Binarized 4-layer MLP on 8 Trainium2 NeuronCores.

Math (from the reference):
    h = x.transpose(0,2,1).reshape(8192, 512)          rows = (n, t), cols = f
    for l in 1..3:  h = sign(batchnorm(h @ sign(Wl).T, gl, bl))
    y = ((h @ sign(W4).T) * scale).reshape(16,512,512).transpose(0,2,1)
(The depthwise conv in the reference is dead code - its result is discarded.)

Strategy:
  - Data parallel over the 8192 rows: core c owns rows of batch elements
    n = 2c, 2c+1 (1024 rows).
  - Activations live transposed on chip: [hidden partition, ktile, row] so
    every matmul is lhsT.T @ rhs with both operands in natural layout and
    BN stats are free-dim reductions (bn_stats) + per-partition broadcasts.
  - After each BN+sign the activations are exactly {-1,0,+1}: layers 2-4 run
    in fp8(e4m3) with DoubleRow at exact integer accumulation in fp32 PSUM.
  - Layer 1 runs either as a single float32r pass or as fp16 hi+lo
    (x = hi + lo, residual ~2^-22) - both inside the fp32 round-off envelope
    of the reference.
  - BatchNorm needs full-batch stats: each core computes per-hidden
    (mean, E[x^2]) over its 1024 rows (bn_stats/bn_aggr) and AllGathers the
    8 partials in two halves so the first collective overlaps the second
    half's matmuls.  sign(g*(h-mu)*rsqrt(var+eps)+b) == sign(h - thr) with
    thr = mu - (b/g)*sqrt(var+eps) for g > 0.
  - Weights are binarized + laid out host-side (layout prep only touches
    sign/transpose/dtype).
"""

import numpy as np
import ml_dtypes

import concourse.bacc as bacc
import concourse.tile as tile
import concourse.mybir as mybir
from concourse.bass_utils import run_bass_kernel_spmd

# ---- problem constants (hardcoded per the grading contract) ----
NB, F, T, H = 16, 512, 512, 2048
CORES = 8
NPC = NB // CORES          # batch elements per core = 2
RPC = NPC * T              # rows per core = 1024
C = RPC // 512             # 512-row chunks per core = 2
KT1 = F // 128             # k-tiles layer 1 = 4
KT = H // 128              # k-tiles layers 2-4 = 16
MT_H = H // 128            # out tiles layers 1-3 = 16
MT_F = F // 128            # out tiles layer 4 = 4
HALF = MT_H // 2           # BN stat-exchange half = 8
EPS = 1e-5

F32 = mybir.dt.float32
F32R = mybir.dt.float32r
BF16 = mybir.dt.bfloat16
FP16 = mybir.dt.float16
FP8 = mybir.dt.float8e4
FP8_NP = ml_dtypes.float8_e4m3
SQRT = mybir.ActivationFunctionType.Sqrt
IDENT = mybir.ActivationFunctionType.Identity
IS_GE = mybir.AluOpType.is_ge
MULT = mybir.AluOpType.mult
DR = mybir.MatmulPerfMode.DoubleRow

L1_MODE = "fp32r"          # "fp32r" (1 pass) or "fp16x2" (hi+lo, 2 passes)

_CACHE = {}


def _build_nc(n_cores=CORES, with_cc=True, reps=1):
    nc = bacc.Bacc(
        "TRN2",
        target_bir_lowering=False,
        debug=False,
        enable_asserts=False,
        num_devices=n_cores,
    )
    if L1_MODE == "fp32r":
        x_dts = [F32R]
        x_names = ["xf"]
        w1_dt = F32R
    else:
        x_dts = [FP16, FP16]
        x_names = ["xhi", "xlo"]
        w1_dt = FP16
    x_d = [
        nc.dram_tensor(nm, [128, KT1, C, 512], dt, kind="ExternalInput")
        for nm, dt in zip(x_names, x_dts)
    ]
    w1_d = nc.dram_tensor("w1q", [MT_H, 128, KT1, 128], w1_dt, kind="ExternalInput")
    w2_d = nc.dram_tensor("w2q", [MT_H, 128, KT, 128], FP8, kind="ExternalInput")
    w3_d = nc.dram_tensor("w3q", [MT_H, 128, KT, 128], FP8, kind="ExternalInput")
    w4_d = nc.dram_tensor("w4q", [MT_F, 128, KT, 128], FP8, kind="ExternalInput")
    bg_d = nc.dram_tensor("bgq", [128, 3, KT], F32, kind="ExternalInput")
    sc_d = nc.dram_tensor("scq", [128, MT_F], F32, kind="ExternalInput")
    rb_d = nc.dram_tensor("rbq", [128, MT_F], F32, kind="ExternalInput")
    y_d = nc.dram_tensor("y", [128, MT_F, C, 512], F32, kind="ExternalOutput")

    with tile.TileContext(nc) as tc:
        with (
            tc.tile_pool(name="persist", bufs=1) as sb,
            tc.tile_pool(name="wts", bufs=6) as wpool,
            tc.tile_pool(name="psum", bufs=8, space="PSUM") as pp,
            tc.tile_pool(name="dram", bufs=1, space="DRAM") as dp,
        ):
            # persistent SBUF tensors
            xs = [
                sb.tile([128, KT1, C, 512], dt, name=f"x{i}")
                for i, dt in enumerate(x_dts)
            ]
            hpre = sb.tile([128, KT, C, 512], F32, name="hpre")
            act_a = sb.tile([128, KT, C, 512], FP8, name="act_a")
            act_b = sb.tile([128, KT, C, 512], FP8, name="act_b")
            stats6 = sb.tile([128, KT, C, 6], F32, name="stats6")
            locms = sb.tile([128, KT, 2], F32, name="locms")
            part = sb.tile([128, 2, KT], F32, name="part")
            bgs = sb.tile([128, 3, KT], F32, name="bgs")
            scs = sb.tile([128, MT_F], F32, name="scs")
            rbs = sb.tile([128, MT_F], F32, name="rbs")
            yout = sb.tile([128, MT_F, C, 512], F32, name="yout")
            epst = sb.tile([128, 1], F32, name="epst")
            ex = sb.tile([128, KT], F32, name="ex")
            exx = sb.tile([128, KT], F32, name="exx")
            var = sb.tile([128, KT], F32, name="var")
            std = sb.tile([128, KT], F32, name="std")
            pthr = sb.tile([128, KT], F32, name="pthr")
            nc.vector.memset(epst[:], EPS)

            for xt, xd in zip(xs, x_d):
                nc.sync.dma_start(xt[:], xd.ap())
            nc.sync.dma_start(bgs[:], bg_d.ap())
            nc.sync.dma_start(scs[:], sc_d.ap())
            nc.sync.dma_start(rbs[:], rb_d.ap())

            def mm_layer(w_dram, wdt, wkt, mt, rhs_list, dr):
                """Matmuls of one layer; fills hpre[:, :mt] and stats6[:, :mt]."""
                for m in range(mt):
                    wt = wpool.tile([128, wkt, 128], wdt, name="wt", tag="wt")
                    nc.sync.dma_start(wt[:], w_dram.ap()[m])
                    for c in range(C):
                        ps = pp.tile([128, 512], F32, name="ps", tag="ps")
                        if dr:
                            nk = wkt // 2
                            for kk in range(nk):
                                nc.tensor.matmul(
                                    ps[:],
                                    lhsT=wt[:, 2 * kk : 2 * kk + 2, :],
                                    rhs=rhs_list[0][:, 2 * kk : 2 * kk + 2, c, :],
                                    start=(kk == 0),
                                    stop=(kk == nk - 1),
                                    perf_mode=DR,
                                )
                        else:
                            ns = wkt * len(rhs_list)
                            s = 0
                            for kk in range(wkt):
                                for rhs in rhs_list:
                                    nc.tensor.matmul(
                                        ps[:],
                                        lhsT=wt[:, kk, :],
                                        rhs=rhs[:, kk, c, :],
                                        start=(s == 0),
                                        stop=(s == ns - 1),
                                    )
                                    s += 1
                        nc.vector.bn_stats(stats6[:, m, c, :], ps[:])
                        nc.scalar.copy(hpre[:, m, c, :], ps[:])

            def bn_sign(li, acts_out, rep=0):
                """Cross-core BN stats + sign, in two halves of 8 hidden
                k-tiles so the first AllGather overlaps the second half's
                matmuls.  Reads hpre/stats6, writes acts_out."""
                for h in range(2):
                    msl = slice(h * HALF, (h + 1) * HALF)
                    for m in range(h * HALF, (h + 1) * HALF):
                        nc.vector.bn_aggr(
                            locms[:, m, :],
                            stats6[:, m, :, :].rearrange("p a b -> p (a b)"),
                        )
                    tmp8 = sb.tile([128, HALF], F32, name=f"tmp8_{li}_{h}",
                                   tag="tmp8")
                    nc.vector.tensor_copy(part[:, 0, msl], locms[:, msl, 0])
                    nc.vector.tensor_mul(tmp8[:], locms[:, msl, 0],
                                         locms[:, msl, 0])
                    nc.vector.tensor_add(part[:, 1, msl], locms[:, msl, 1],
                                         tmp8[:])
                    if with_cc:
                        ccin = dp.tile([128, 2 * HALF], F32,
                                       name=f"ccin{li}_{h}_{rep}")
                        ccout = dp.tile(
                            [n_cores * 128, 2 * HALF], F32,
                            name=f"ccout{li}_{h}_{rep}", addr_space="Shared",
                        )
                        nc.sync.dma_start(
                            ccin[:].rearrange("p (a b) -> p a b", a=2),
                            part[:, :, msl],
                        )
                        nc.gpsimd.collective_compute(
                            "AllGather",
                            mybir.AluOpType.bypass,
                            replica_groups=[list(range(n_cores))],
                            ins=[ccin.opt()],
                            outs=[ccout.opt()],
                        )
                        gath = sb.tile([128, n_cores, 2 * HALF], F32,
                                       name=f"gath_{li}_{h}", tag="gath")
                        nc.sync.dma_start(
                            gath[:], ccout[:].rearrange("(r p) n -> p r n", p=128)
                        )
                        t4 = sb.tile([128, 4, 2 * HALF], F32,
                                     name=f"t4_{li}_{h}", tag="t4")
                        t2 = sb.tile([128, 2, 2 * HALF], F32,
                                     name=f"t2_{li}_{h}", tag="t2")
                        t1 = sb.tile([128, 2 * HALF], F32,
                                     name=f"t1_{li}_{h}", tag="t1")
                        nc.vector.tensor_add(t4[:], gath[:, 0:4, :],
                                             gath[:, 4:8, :])
                        nc.vector.tensor_add(t2[:], t4[:, 0:2, :],
                                             t4[:, 2:4, :])
                        nc.vector.tensor_add(t1[:], t2[:, 0, :], t2[:, 1, :])
                        srcmean, srcexx = t1[:, 0:HALF], t1[:, HALF : 2 * HALF]
                        inv = 1.0 / n_cores
                    else:
                        srcmean, srcexx = part[:, 0, msl], part[:, 1, msl]
                        inv = 1.0
                    nc.vector.tensor_scalar_mul(ex[:, msl], srcmean, inv)
                    nc.vector.tensor_scalar_mul(exx[:, msl], srcexx, inv)
                    nc.vector.tensor_mul(var[:, msl], ex[:, msl], ex[:, msl])
                    nc.vector.tensor_sub(var[:, msl], exx[:, msl], var[:, msl])
                    nc.vector.tensor_scalar_max(var[:, msl], var[:, msl], 0.0)
                    nc.scalar.activation(std[:, msl], var[:, msl], SQRT,
                                         bias=epst[:])
                    nc.vector.tensor_mul(std[:, msl], bgs[:, li, msl],
                                         std[:, msl])
                    # pthr = mu - (b/g)*sqrt(var+eps); activations become
                    # 2*[h >= pthr] in {0,2} - a +1 shift of sign(h-pthr)
                    # per hidden unit, which training-mode BN of the next
                    # layer absorbs (stats are shift-invariant); layer 4
                    # corrects via the host-computed rowsum bias.
                    nc.vector.tensor_sub(pthr[:, msl], ex[:, msl], std[:, msl])
                    for m in range(h * HALF, (h + 1) * HALF):
                        nc.vector.tensor_scalar(
                            acts_out[:, m, :, :], hpre[:, m, :, :],
                            pthr[:, m : m + 1], 2.0,
                            op0=IS_GE, op1=MULT,
                        )

            for rep in range(reps):
                # layer 1 -> BN1 -> act_a
                mm_layer(w1_d, w1_dt, KT1, MT_H, xs, dr=False)
                bn_sign(0, act_a, rep)
                # layer 2 (fp8) -> BN2 -> act_b
                mm_layer(w2_d, FP8, KT, MT_H, [act_a], dr=True)
                bn_sign(1, act_b, rep)
                # layer 3 (fp8) -> BN3 -> act_a
                mm_layer(w3_d, FP8, KT, MT_H, [act_b], dr=True)
                bn_sign(2, act_a, rep)
                # layer 4 (fp8) + per-feature scale
                for m in range(MT_F):
                    wt = wpool.tile([128, KT, 128], FP8, name="wt4", tag="wt")
                    nc.sync.dma_start(wt[:], w4_d.ap()[m])
                    for c in range(C):
                        ps = pp.tile([128, 512], F32, name="ps4", tag="ps")
                        for kk in range(KT // 2):
                            nc.tensor.matmul(
                                ps[:],
                                lhsT=wt[:, 2 * kk : 2 * kk + 2, :],
                                rhs=act_a[:, 2 * kk : 2 * kk + 2, c, :],
                                start=(kk == 0),
                                stop=(kk == KT // 2 - 1),
                                perf_mode=DR,
                            )
                        nc.scalar.activation(
                            yout[:, m, c, :], ps[:], IDENT,
                            bias=rbs[:, m : m + 1], scale=scs[:, m : m + 1],
                        )
                nc.sync.dma_start(y_d.ap(), yout[:])

    nc.compile()
    return nc


def _get_nc():
    if "nc" not in _CACHE:
        _CACHE["nc"] = _build_nc()
    return _CACHE["nc"]


def _wq(W, np_dt):
    """sign(W).T laid out [mt, 128, kt, 128] = (out tile, in%128, in//128, out%128)."""
    Wt = np.sign(np.asarray(W, np.float32)).T
    IN, OUT = Wt.shape
    kt, mt = IN // 128, OUT // 128
    return np.ascontiguousarray(
        Wt.reshape(kt, 128, mt, 128).transpose(2, 1, 0, 3).astype(np_dt)
    )


def _xq(a, c):
    """per-core x slice -> [128, KT1, NPC, 512]"""
    s = a[NPC * c : NPC * (c + 1)]  # (2, 512, 512) = (n, f, t)
    return np.ascontiguousarray(s.reshape(NPC, KT1, 128, T).transpose(2, 1, 0, 3))


def _prep_in_maps(inputs):
    x = np.asarray(inputs["x"], np.float32)
    if L1_MODE == "fp32r":
        xparts = {"xf": x}
        w1q = _wq(inputs["W1"], np.float32)
    else:
        xhi = x.astype(np.float16)
        xlo = (x - xhi.astype(np.float32)).astype(np.float16)
        xparts = {"xhi": xhi, "xlo": xlo}
        w1q = _wq(inputs["W1"], np.float16)

    w2q = _wq(inputs["W2"], FP8_NP)
    w3q = _wq(inputs["W3"], FP8_NP)
    w4q = _wq(inputs["W4"], FP8_NP)

    def _pk(v):  # (2048,) -> [128, 16]
        return np.ascontiguousarray(np.asarray(v, np.float32).reshape(KT, 128).T)

    bgq = np.stack(
        [
            _pk(np.where(inputs[g] != 0, inputs[b] / inputs[g], 0.0))
            for g, b in (("g1", "b1"), ("g2", "b2"), ("g3", "b3"))
        ],
        axis=1,
    ).astype(np.float32)
    bgq = np.ascontiguousarray(bgq)
    scale = np.asarray(inputs["scale"], np.float32)
    scq = np.ascontiguousarray(scale.reshape(MT_F, 128).T)
    # layer-4 bias correcting the {0,2} activation encoding:
    # y = (W4b @ a - rowsum(W4b)) * scale, rowsum folded into the ACT bias
    rs4 = np.sign(np.asarray(inputs["W4"], np.float32)).sum(axis=1)
    rbq = np.ascontiguousarray(
        (-rs4 * scale).astype(np.float32).reshape(MT_F, 128).T
    )

    in_maps = []
    for c in range(CORES):
        m = {nm: _xq(arr, c) for nm, arr in xparts.items()}
        m.update(w1q=w1q, w2q=w2q, w3q=w3q, w4q=w4q, bgq=bgq, scq=scq,
                 rbq=rbq)
        in_maps.append(m)
    return in_maps


def _assemble(results):
    y = np.empty((NB, F, T), np.float32)
    for c in range(CORES):
        r = results[c]["y"]  # [128, MT_F, C, 512]
        y[NPC * c : NPC * (c + 1)] = r.transpose(2, 1, 0, 3).reshape(NPC, F, T)
    return y


def _valid(y, inputs):
    """Catches the (rare) garbage first execution after NEFF load: outputs
    are sums of <=2048 terms of +-1 times scale, so any non-finite value or
    magnitude above that bound means the run must be retried."""
    bound = 2048.0 * max(1.0, float(np.abs(inputs["scale"]).max())) * 1.001
    return np.isfinite(y).all() and float(np.abs(y).max()) <= bound


def kernel(**inputs):
    nc = _get_nc()
    in_maps = _prep_in_maps(inputs)
    for _ in range(3):
        res = run_bass_kernel_spmd(nc, in_maps, core_ids=list(range(CORES)))
        y = _assemble(res.results)
        if _valid(y, inputs):
            return y
    return y


if __name__ == "__main__":
    rng = np.random.default_rng(0)
    ins = dict(
        x=rng.standard_normal((NB, F, T)).astype(np.float32),
        conv_w=rng.standard_normal((F, 1, 5)).astype(np.float32),
        W1=rng.standard_normal((H, F)).astype(np.float32),
        g1=np.ones(H, np.float32), b1=np.zeros(H, np.float32),
        W2=rng.standard_normal((H, H)).astype(np.float32),
        g2=np.ones(H, np.float32), b2=np.zeros(H, np.float32),
        W3=rng.standard_normal((H, H)).astype(np.float32),
        g3=np.ones(H, np.float32), b3=np.zeros(H, np.float32),
        W4=rng.standard_normal((F, H)).astype(np.float32),
        scale=np.ones(F, np.float32),
    )
    out = kernel(**ins)
    print(out.shape, out.dtype)
